# revision 2
# baseline (speedup 1.0000x reference)
"""NT-Xent contrastive loss (forward) on 8 TRN2 NeuronCores via Bass/Tile.

Math: with h = concat(h_i, h_j) [N=8192, D=256], sim = (h @ h.T) / 0.5,
loss = mean_r( logsumexp_j(sim[r, j], j != r) - pos_r ), where
pos_r = sim[r, partner(r)] = 2 * h_i[q] . h_j[q].  The loss separates:
loss = (sum_r lse_r - 4 * sum(h_i * h_j)) / N; the pos term is a single
1M-element dot the host computes exactly in float64.

Sharding: core c owns rows [1024c, 1024c + 1024).  Each core receives the
full transposed h, column-rotated by its row offset, so one SPMD program
serves all 8 cores: the self-similarity diagonal lands at core-invariant
positions.

Per core: the PE builds each 128-row block of sim in PSUM from fp8 e4m3
operands with the DoubleRow perf mode (2 fp8 MACs per cell per cycle,
fp32 accumulate); the whole 8-bank PSUM is one hand-managed ring tile of
16 x 512-column bank chunks per row-block.  Per half row-block the DVE
evaluates a u16/bfloat16 Schraudolph bit-trick exp over a 1536-column
chunk (2 passes: tensor_scalar to u16 bits, reduce over the bf16 view)
while the scalar engine applies exp(2x - M_row) with a fused row-sum
(accum_out) over a 2560-column chunk; the diagonal is masked by
accumulating I.T @ (-1e9 shifted-diag) as an extra bf16 matmul into the
DVE chunk.  Each core emits a [128, 32] tile of partial sums; the host
finishes with log/sum in float64.  M is a runtime input (per-row); if a
row's exp-sum under/overflows fp32, the host retries with a shifted M
for those rows.
"""

import numpy as np
import ml_dtypes

B = 4096
D = 256
N = 2 * B            # 8192 rows/cols of sim
NCORES = 8
RPC = N // NCORES    # 1024 rows per core
KCH = D // 128       # 2 contraction chunks of 128
NRB = RPC // 128     # 8 row-blocks of 128 per core
M_DEFAULT = 161.0    # logsumexp shift; safe while rowmax(2*h@h.T) in [M-70, M+79]
MASK_NEG = -1.0e9

# Schraudolph fast-exp constants for bf16 bit patterns
# (exp(y) ~= bitcast_bf16(u16(round(A16*y + B16)))); B16 calibrated so the
# phase-averaged, exp-weighted relative error is ~0 (per-term max +-4%).
# Negative overflow saturates the u16 convert at 0 == +0.0bf16 which sums
# as zero; the diagonal's -1e9 mask lands there.
EXP_A16 = float(2 ** 7 / np.log(2.0))
EXP_B16 = 16248.5537

# Per-row-block column split (after rotation).  D* chunks go to the DVE
# fast-exp path, A* chunks to the scalar engine's exact exp.  The diagonal
# (columns 128rb < 1024) lies in D1.  Chunk = one 512-col PSUM bank; the
# 16 banks of a row-block map onto the 8 physical banks twice (ring).
D1 = (0, 1536)       # banks 0-2 of half 1
A1 = (1536, 4096)    # banks 3-7 of half 1
D2 = (4096, 5632)    # banks 0-2 of half 2
A2 = (5632, 8192)    # banks 3-7 of half 2

TRACE = False        # set True (e.g. from test.py) to request an NTFF trace
LAST_RESULTS = None  # BassKernelResults of the last run (for profiling)

_cache = {}


def _build():
    """Build the SPMD Bass/Tile program once per process."""
    if "nc" in _cache:
        return _cache["nc"]

    import concourse.tile as tile
    import concourse.mybir as mybir
    from concourse import bacc

    f32 = mybir.dt.float32
    bf16 = mybir.dt.bfloat16
    fp8 = mybir.dt.float8e4
    u16 = mybir.dt.uint16
    DR = mybir.MatmulPerfMode.DoubleRow

    nc = bacc.Bacc("TRN2", target_bir_lowering=False, num_devices=NCORES)
    ht_dram = nc.dram_tensor("ht", [KCH, 128, N], fp8, kind="ExternalInput").ap()
    # eye[0][0] = I [128,128]; mask[v] [128,512] holds -1e9 at [p, 128v+p].
    # I.T @ mask[v] accumulated into a sim-block 512-slice masks its diagonal.
    eye_dram = nc.dram_tensor("eye", [1, 128, 128], bf16, kind="ExternalInput").ap()
    maskr_dram = nc.dram_tensor("maskr", [128, 4, 512], bf16, kind="ExternalInput").ap()
    bias_dram = nc.dram_tensor("biasm", [128, NRB], f32, kind="ExternalInput").ap()
    bias2_dram = nc.dram_tensor("bias2", [128, NRB], f32, kind="ExternalInput").ap()
    out_dram = nc.dram_tensor("out", [128, 32], f32, kind="ExternalOutput").ap()

    with tile.TileContext(nc) as tc:
        with (
            tc.tile_pool(name="hpool", bufs=1) as hpool,
            tc.tile_pool(name="small", bufs=1) as small,
            tc.tile_pool(name="ipool", bufs=3) as ipool,
            tc.tile_pool(name="psum", bufs=1, space="PSUM") as psum,
        ):
            # Small constants go on the gpsimd (SWDGE) queue so they land
            # while the sync queue streams the big h.T chunks.
            eye_pos = small.tile([128, 128], bf16)
            nc.gpsimd.dma_start(out=eye_pos, in_=eye_dram[0])
            maskr_sb = small.tile([128, 4, 512], bf16)
            nc.gpsimd.dma_start(out=maskr_sb, in_=maskr_dram)
            bias_sb = small.tile([128, NRB], f32)
            nc.gpsimd.dma_start(out=bias_sb, in_=bias_dram)
            bias2_sb = small.tile([128, NRB], f32)
            nc.gpsimd.dma_start(out=bias2_sb, in_=bias2_dram)

            # Warm the ACT exp table (~2.7us load) during the DMA prologue so
            # the first real exp doesn't pay for it.
            warm_sb = small.tile([128, 1], f32)
            nc.scalar.activation(
                out=warm_sb, in_=bias_sb[:, 0:1],
                func=mybir.ActivationFunctionType.Exp, bias=0.0, scale=0.0,
            )

            # The whole 8-bank PSUM as one ring tile of [128, 8, 512] f32.
            ring = psum.tile([128, 8, 512], f32, name="ring")

            # Warm the PE's HAM clock gate (cold = 1.2GHz for the first
            # ~3.4us of activity) with dummy matmuls on a memset tile while
            # the h.T DMAs are still in flight.
            wsrc = small.tile([128, 128], bf16)
            nc.vector.memset(wsrc, 0.0)
            for w in range(32):
                nc.tensor.matmul(
                    ring[:, w % 2, 0:128],
                    lhsT=wsrc, rhs=wsrc,
                    start=True, stop=True,
                )

            # h.T in SBUF on the sync HWDGE queue, in the order compute
            # consumes it.  Each DMA carries BOTH contraction halves of a
            # column range (tile layout [128, 2, width]) so the DoubleRow
            # matmul reads the two k-planes side by side.
            col_ranges = [(0, 1024), (1024, 2560), (2560, 4096),
                          (4096, 6656), (6656, 8192)]
            ht_tiles = []
            for di, (c0, c1) in enumerate(col_ranges):
                t = hpool.tile([128, KCH, c1 - c0], fp8, name=f"ht_{c0}")
                nc.sync.dma_start(
                    out=t,
                    in_=ht_dram[:, :, c0:c1].rearrange("k p c -> p k c"),
                )
                ht_tiles.append(t)

            def rhs_slice(c0, w=512):
                """[128, 2, w] slice of rotated h.T at global column c0."""
                for (r0, r1), t in zip(col_ranges, ht_tiles):
                    if r0 <= c0 < r1:
                        assert c0 + w <= r1
                        return t[:, :, c0 - r0:c0 - r0 + w]
                raise AssertionError(c0)

            res_sb = small.tile([128, 32], f32)

            for rb in range(NRB):
                lhsT = ht_tiles[0][:, :, rb * 128:(rb + 1) * 128]
                dm = (128 * rb) // 512      # D1 bank holding the diagonal
                # Diagonal mask first (bf16 eye weights), then all 16 fp8
                # DoubleRow chunks with this row-block's weights: 2 weight
                # loads per row-block.
                nc.tensor.matmul(
                    ring[:, dm, :],
                    lhsT=eye_pos,
                    rhs=maskr_sb[:, rb % 4, :],
                    start=True,
                    stop=False,
                )
                for half, (d0, a0) in enumerate(((D1[0], A1[0]), (D2[0], A2[0]))):
                    for b in range(8):
                        c0 = (d0 if b < 3 else a0 - 1536) + b * 512
                        bank = b if half == 0 else b
                        is_dm = half == 0 and b == dm
                        nc.tensor.matmul(
                            ring[:, bank, :],
                            lhsT=lhsT,
                            rhs=rhs_slice(c0),
                            start=not is_dm,
                            stop=True,
                            perf_mode=DR,
                        )
                    # consumers for this half
                    ti = ipool.tile([128, 1536], u16, name="ti")
                    nc.vector.tensor_scalar(
                        ti, ring[:, 0:3, :], 2.0 * EXP_A16,
                        bias2_sb[:, rb:rb + 1],
                        mybir.AluOpType.mult, mybir.AluOpType.add,
                    )
                    nc.vector.reduce_sum(
                        res_sb[:, rb * 4 + 2 * half + 1:rb * 4 + 2 * half + 2],
                        ti.bitcast(bf16),
                        axis=mybir.AxisListType.X,
                    )
                    nc.scalar.activation(
                        out=ring[:, 3:8, :],
                        in_=ring[:, 3:8, :],
                        func=mybir.ActivationFunctionType.Exp,
                        bias=bias_sb[:, rb:rb + 1],
                        scale=2.0,
                        accum_out=res_sb[:, rb * 4 + 2 * half:rb * 4 + 2 * half + 1],
                    )

            # Ship rb0-6 partials while rb7 is still computing; only a
            # tiny transfer remains on the kernel tail.
            nc.sync.dma_start(out=out_dram[:, 0:28], in_=res_sb[:, 0:28])
            nc.sync.dma_start(out=out_dram[:, 28:32], in_=res_sb[:, 28:32])

    nc.compile()
    _cache["nc"] = nc
    return nc


def _make_static_inputs(h_i, h_j):
    """Per-core rotated h.T (fp8 e4m3) plus the diag mask (shared)."""
    h = np.concatenate([np.asarray(h_i), np.asarray(h_j)], axis=0).astype(np.float32)
    hT = np.ascontiguousarray(h.T)  # [256, 8192]
    np.clip(hT, -240.0, 240.0, out=hT)
    hts = []
    for c in range(NCORES):
        htc = np.roll(hT, -RPC * c, axis=1)
        hts.append(
            np.ascontiguousarray(
                htc.astype(ml_dtypes.float8_e4m3).reshape(KCH, 128, N)
            )
        )
    eye = np.zeros((1, 128, 128), dtype=ml_dtypes.bfloat16)
    p = np.arange(128)
    eye[0, p, p] = 1.0
    maskr = np.zeros((128, 4, 512), dtype=ml_dtypes.bfloat16)
    for v in range(4):
        maskr[p, v, 128 * v + p] = MASK_NEG
    return hts, eye, maskr


def _axon_reset():
    """Recover the axon-tunneled NeuronCores if a previous process left them
    in an unrecoverable state."""
    try:
        import ctypes

        lib = ctypes.CDLL("/opt/axon/libaxon_pjrt.so")
        lib.axon_reset.restype = ctypes.c_int64
        return lib.axon_reset() == 0
    except Exception:
        return False


def _run(nc, hts, eye, maskr, M_per_core):
    global LAST_RESULTS
    from concourse import bass_utils

    in_maps = [
        {
            "ht": hts[c],
            "eye": eye,
            "maskr": maskr,
            "biasm": (-M_per_core[c]).astype(np.float32),
            "bias2": (EXP_B16 - EXP_A16 * M_per_core[c]).astype(np.float32),
        }
        for c in range(NCORES)
    ]
    try:
        results = bass_utils.run_bass_kernel_spmd(
            nc, in_maps, core_ids=list(range(NCORES)), trace=TRACE
        )
    except Exception:
        # A wedged accelerator (e.g. NRT_EXEC_UNIT_UNRECOVERABLE from an
        # earlier crashed process) survives process restarts; reset and retry.
        if not _axon_reset():
            raise
        results = bass_utils.run_bass_kernel_spmd(
            nc, in_maps, core_ids=list(range(NCORES)), trace=TRACE
        )
    LAST_RESULTS = results
    return results.results


def kernel(h_i, h_j):
    nc = _build()
    hts, eye, maskr = _make_static_inputs(h_i, h_j)

    # Per-core, per-row logsumexp shift M (as the activation bias -M).
    M = [np.full((128, NRB), M_DEFAULT, dtype=np.float64) for _ in range(NCORES)]

    lse = [np.full((128, NRB), np.nan) for _ in range(NCORES)]

    for attempt in range(4):
        res = _run(nc, hts, eye, maskr, M)
        any_bad = False
        for c in range(NCORES):
            out = res[c]["out"].astype(np.float64)
            S = out.reshape(128, NRB, 4).sum(axis=2)
            good = np.isfinite(S) & (S > 0.0)
            upd = good & ~np.isfinite(lse[c])
            lse[c][upd] = M[c][upd] + np.log(S[upd])
            bad = ~np.isfinite(lse[c])
            if bad.any():
                any_bad = True
                # S == 0 -> M too high for those rows; S inf/nan -> too low.
                over = bad & ~np.isfinite(S)
                under = bad & ~over
                M[c][under] -= 75.0
                M[c][over] += 75.0
        if not any_bad:
            break

    total_lse = sum(l.sum() for l in lse)
    # sum_r pos_r = 4 * sum(h_i * h_j), computed exactly on the host.
    total_pd = float(
        np.sum(np.asarray(h_i, dtype=np.float64) * np.asarray(h_j, dtype=np.float64))
    )
    loss = (total_lse - 4.0 * total_pd) / float(N)
    return np.array(loss, dtype=np.float32)


if __name__ == "__main__":
    # Smoke test with random data (not the reference inputs).
    rng = np.random.default_rng(0)
    h_i = rng.standard_normal((B, D), dtype=np.float32)
    h_j = rng.standard_normal((B, D), dtype=np.float32)
    print("loss:", kernel(h_i, h_j))
